# revision 52
# baseline (speedup 1.0000x reference)
"""CP-decomposed embedding lookup kernel for Trainium2 (8 NeuronCores).

Math (matches the CPEmbedding reference):
    A = khatri_rao(U0, U1, U2)            # [500000, 32]
    B = khatri_rao(V0, V1)                # [128, 32]
    out = (A @ B.T)[x]                    # [1024, 200, 128]

Per lookup x = a*5000 + b*50 + c:
    w[r]   = U0[a, r] * U1[b, r] * U2[c, r]
    out[x] = w @ B.T

Instead of per-row DMA gathers (whose SWDGE descriptor generation serializes
on the Q7/Pool engine at ~8 ns/row -> 410 us/core), the factor gathers are
computed as one-hot matmuls on the idle Tensor engine:

    oh_a[v, i] = (a_i == v)   (bf16, exact)     G0T = U0.T @ oh_a  [32, n]
    wT = G0T * G1T * G2T  (DVE elementwise)     out = wT.T @ B.T

Index delivery: the host replicates packed u16 index planes down the
partition axis (p = a + 256*b on 100 partitions), and the a/b one-hot
compares run as DVE tensor_scalar passes from SBUF at 2-byte dtype (fast
DVE mode), with no PSUM broadcast needed. The DVE ISA cannot mix bitwise
and arithmetic ops in one pass (and has no mod), so the field extraction
is a separate bitwise_and pass before each is_equal. The (tiny) c one-hot
plane is built directly on the host - same bytes as replicated c values.

Packing: rank is only 32, so four 512-lookup "packs" share each PSUM bank
at partition offsets 0/32/64/96 (weights loaded at PE array column offsets
via tile_position). The two Khatri-Rao products then run as single
[128, 512] DVE ops covering 2048 lookups each.

c-pairing: the c factor has only 50 vocab rows, so TWO packs' U2 gathers
run as ONE K=100 matmul with a block-diagonal [100, 64] weight (top rows =
U2 for the even pack at rank cols 0-32, bottom rows = U2 for the odd pack
at 32-64). The host lays the replicated c-plane with the partner pack's
c+50 in rows 50-99 so one is_equal against iota(100) produces the stacked
pair one-hot directly.

Output: the final contraction runs TRANSPOSED — one N=512 matmul per pack
(lhsT = the constant B.T rank-block, rhs = the wT slice) producing
[emb, lookups] in PSUM, so each pack needs 1 output matmul instead of 4
and one ldweights of the constant. DRAM output is [128, 25600] bf16
(emb-major); the host transposes and upcasts during assembly.

Sharding: CP factors replicated; the 204800 lookups are split evenly across
the 8 cores (each computes a contiguous [25600, 128] slice of the output).
"""

import ml_dtypes
import numpy as np

import concourse.bacc as bacc
import concourse.bass as bass
import concourse.mybir as mybir
import concourse.tile as tile
from concourse import bass_utils

# Problem constants (hardcoded per the harness contract).
VOC = (100, 100, 50)  # a, b, c
RANK = 32
E = 128  # emb = 8 * 16
N_CORES = 8
X_SHAPE = (1024, 200)
N_TOTAL = X_SHAPE[0] * X_SHAPE[1]  # 204800
N_CORE = N_TOTAL // N_CORES  # 25600
P = 128

PACK = 512  # lookups per pack (one PSUM-bank column span at fp32)
# supers: groups of packs processed per pipeline stage. Small ramp-up supers
# let the PE start after ~150 KB of index DMA instead of 613 KB; a small
# tail super shortens the copy/DMA drain. 50 packs total = 25600 lookups.
SUPERS = [1, 1, 2] + [4] * 11 + [1, 1]  # packs per super
assert sum(SUPERS) * PACK == N_CORE
# c-pair columns: one 512-col block per pack PAIR (odd tail pack gets a
# half-empty block whose bottom rows match nothing).
PAIRS = [(sp + 1) // 2 for sp in SUPERS]
PAIR_COLS = sum(PAIRS) * PACK

F32 = mybir.dt.float32
BF16 = mybir.dt.bfloat16
U16 = mybir.dt.uint16

AND = mybir.AluOpType.bitwise_and
EQ = mybir.AluOpType.is_equal
MULT = mybir.AluOpType.mult


def build_program():
    nc = bacc.Bacc("TRN2", target_bir_lowering=False, debug=False)

    # ---- DRAM I/O (per core) ----
    abrep_d = nc.dram_tensor("abrep", [VOC[0], N_CORE], U16, kind="ExternalInput")
    crep_d = nc.dram_tensor("crep", [VOC[0], PAIR_COLS], BF16, kind="ExternalInput")
    # all factor/iota constants packed in one tensor -> ONE startup DMA:
    # cols 0-31 u0, 32-63 u1, 64-127 u2pair, 128-255 btb4, 256 iota_a,
    # 257 iota_b (= iota_a).
    consts_d = nc.dram_tensor("consts", [P, 258], F32, kind="ExternalInput")
    out_d = nc.dram_tensor("out", [P, N_CORE], BF16, kind="ExternalOutput")

    with tile.TileContext(nc) as tc:
        const = tc.alloc_tile_pool(name="const", bufs=1)

        # ---------- one-time setup ----------
        cf = const.tile([P, 258], F32)
        nc.sync.dma_start(cf[:], consts_d.ap())
        cb = const.tile([P, 256], BF16)
        nc.vector.tensor_copy(cb[:], cf[:][:, 0:256])
        u0b = cb[:][0 : VOC[0], 0:32]
        u1b = cb[:][0 : VOC[1], 32:64]
        u2b = cb[:][0 : VOC[0], 64:128]
        btb = cb[:][:, 128:256]
        iota_a = cf[:][0 : VOC[0], 256:257]
        iota_b = cf[:][0 : VOC[0], 257:258]

        # ---------- pools ----------
        idxp = tc.alloc_tile_pool(name="idx", bufs=4)
        extp = tc.alloc_tile_pool(name="ext", bufs=3)
        ohp = tc.alloc_tile_pool(name="oh", bufs=3)
        wp = tc.alloc_tile_pool(name="w", bufs=2)
        wtp = tc.alloc_tile_pool(name="wt", bufs=2)
        osp = tc.alloc_tile_pool(name="os", bufs=3)
        # PSUM: 3 single-buffered G banks (the DVE products drain them
        # early) + 4 rotating out banks so the per-pack output matmuls never
        # wait on the ACT copies = 7 of 8 banks
        gp = tc.alloc_tile_pool(name="g", bufs=1, space="PSUM")
        op = tc.alloc_tile_pool(name="o", bufs=5, space="PSUM")

        MAXS = max(SUPERS) * PACK

        # Per-super state carried between loop iterations for 1-deep
        # software pipelining (o-matmuls of super s emitted after the G
        # matmuls of super s+1 so the PE never waits on the DVE products).
        pend = None  # (wt_tile, n_packs, row0)

        def emit_back_end(pend):
            wt, sp, col0 = pend
            # one SBUF staging tile and ONE output DMA per super: each
            # dma_start costs ~700ns of serial SP-sequencer issue time, so
            # merging 4 per-pack stores into one is a direct wall-clock win.
            osb = osp.tile([P, 4 * PACK], BF16, tag="osb")
            for p in range(sp):
                ops = op.tile([P, PACK], F32, tag="ops")
                # transposed final contraction: out[e, i] for the whole pack
                # in ONE matmul (constant B.T rank-block as the stationary).
                nc.tensor.matmul(
                    out=ops[:],
                    lhsT=btb[32 * p : 32 * p + 32, :],
                    rhs=wt[:][32 * p : 32 * p + 32, :],
                    start=True,
                    stop=True,
                    tile_position=(32 * p, 0),
                )
                nc.scalar.copy(osb[:][:, p * PACK : (p + 1) * PACK], ops[:])
            nc.sync.dma_start(
                out_d.ap()[:, col0 : col0 + sp * PACK], osb[:][:, 0 : sp * PACK]
            )

        MAXP = max(PAIRS) * PACK
        NS = len(SUPERS)
        offs = [sum(SUPERS[:i]) * PACK for i in range(NS)]
        poffs = [sum(PAIRS[:i]) * PACK for i in range(NS)]

        def emit_front(si):
            """Index DMA + one-hot builds for super si (sync + DVE)."""
            sp = SUPERS[si]
            S = sp * PACK
            S2 = PAIRS[si] * PACK
            off, poff = offs[si], poffs[si]
            abr = idxp.tile([VOC[0], MAXS], U16, tag="abr", name=f"abr{si}")
            # the c one-hot is host-built (same bytes as replicated c values)
            oh_c = ohp.tile([VOC[0], MAXP], BF16, tag="ohc", name=f"ohc{si}")
            nc.sync.dma_start(abr[:][:, 0:S], abrep_d.ap()[:, off : off + S])
            nc.scalar.dma_start(oh_c[:][:, 0:S2], crep_d.ap()[:, poff : poff + S2])

            ta = extp.tile([VOC[0], MAXS], U16, tag="ta", name=f"ta{si}")
            tb = extp.tile([VOC[0], MAXS], U16, tag="tb", name=f"tb{si}")
            nc.vector.tensor_scalar(
                out=ta[:][:, 0:S], in0=abr[:][:, 0:S],
                scalar1=0x00FF, scalar2=None, op0=AND,
            )
            nc.vector.tensor_scalar(
                out=tb[:][:, 0:S], in0=abr[:][:, 0:S],
                scalar1=0xFF00, scalar2=None, op0=AND,
            )
            oh_a = ohp.tile([VOC[0], MAXS], BF16, tag="oha", name=f"oha{si}")
            oh_b = ohp.tile([VOC[1], MAXS], BF16, tag="ohb", name=f"ohb{si}")
            nc.vector.tensor_scalar(
                out=oh_a[:][:, 0:S], in0=ta[:][:, 0:S],
                scalar1=iota_a, scalar2=None, op0=EQ,
            )
            nc.vector.tensor_scalar(
                out=oh_b[:][:, 0:S], in0=tb[:][:, 0:S],
                scalar1=iota_b, scalar2=None, op0=EQ,
            )
            fronts[si] = (
                oh_a[:][:, 0:S],
                oh_b[:][:, 0:S],
                oh_c[:][:, 0:S2],
            )

        def emit_mid(si, front):
            """G matmuls (PE) with the g0 staging copy (ACT) overlapped."""
            sp = SUPERS[si]
            oh_a, oh_b, oh_c = front  # AP views into the group front tiles
            g0 = gp.tile([P, PACK], F32, tag="g0", name=f"g0_{si}")
            g1 = gp.tile([P, PACK], F32, tag="g1", name=f"g1_{si}")
            g2 = gp.tile([P, PACK], F32, tag="g2", name=f"g2_{si}")
            nr = 32 * sp
            s0 = wp.tile([P, PACK], F32, tag="s0", name=f"s0_{si}")
            for p in range(sp):
                nc.tensor.matmul(
                    out=g0[:][32 * p : 32 * p + 32, :],
                    lhsT=u0b, rhs=oh_a[:, p * PACK : (p + 1) * PACK],
                    start=True, stop=True, tile_position=(0, 32 * p),
                )
            # stage g0 -> SBUF on ACT while the PE continues with g1/g2
            # (DVE tensor_tensor may read at most one PSUM operand).
            nc.scalar.copy(s0[:][0:nr, :], g0[:][0:nr, :])
            for p in range(sp):
                nc.tensor.matmul(
                    out=g1[:][32 * p : 32 * p + 32, :],
                    lhsT=u1b, rhs=oh_b[:, p * PACK : (p + 1) * PACK],
                    start=True, stop=True, tile_position=(0, 32 * p),
                )
            for q in range(PAIRS[si]):
                # one K=100 matmul gathers U2 for BOTH packs of the pair
                # (block-diagonal weights; odd tail pair has a zero bottom).
                nc.tensor.matmul(
                    out=g2[:][64 * q : 64 * q + 64, :],
                    lhsT=u2b,
                    rhs=oh_c[:, q * PACK : (q + 1) * PACK],
                    start=True, stop=True, tile_position=(0, 64 * q),
                )
            return g0, g1, g2, s0

        def emit_prods(si, mid):
            sp = SUPERS[si]
            _, g1, g2, s0 = mid
            nr = 32 * sp
            w01 = wp.tile([P, PACK], F32, tag="w01", name=f"w01_{si}")
            wt = wtp.tile([P, PACK], BF16, tag="wt", name=f"wt{si}")
            nc.vector.tensor_tensor(
                out=w01[:][0:nr, :], in0=s0[:][0:nr, :], in1=g1[:][0:nr, :],
                op=MULT,
            )
            nc.vector.tensor_tensor(
                out=wt[:][0:nr, :], in0=w01[:][0:nr, :], in1=g2[:][0:nr, :],
                op=MULT,
            )
            return wt

        # PE warm-up: ~3us of back-to-back junk matmuls during the DMA fill
        # window so the Tensor engine's activity-gated clock ramps to 2.4GHz
        # before the first real matmul (it idles at half clock otherwise).
        for wi in range(7):
            wps = op.tile([P, PACK], F32, tag="ops", name=f"warm{wi}")
            nc.tensor.matmul(
                out=wps[:][:, 0:128], lhsT=cb[:][:, 128:256],
                rhs=cb[:][:, 0:128], start=True, stop=True,
            )

        # Software pipeline: upcoming supers' one-hot builds are queued on
        # the DVE before this super's W-products, so the DVE works ahead
        # while the PE runs G(s) instead of ping-ponging serially.
        fronts = {}
        emit_front(0)
        if NS > 1:
            emit_front(1)
        for si, sp in enumerate(SUPERS):
            if si + 2 < NS:
                emit_front(si + 2)
            mid = emit_mid(si, fronts.pop(si))
            if pend is not None:
                emit_back_end(pend)
            wt = emit_prods(si, mid)
            pend = (wt, sp, offs[si])

        emit_back_end(pend)

        for pool in (op, gp, osp, wtp, wp, ohp, extp, idxp, const):
            pool.release()

    nc.compile()
    return nc


_CACHE: dict = {}


def _get_program():
    if "nc" not in _CACHE:
        _CACHE["nc"] = build_program()
    return _CACHE["nc"]


def make_in_maps(x, U0, U1, U2, V0, V1):
    xf = np.asarray(x).reshape(-1).astype(np.int64)
    a = xf // (VOC[1] * VOC[2])
    b = (xf // VOC[2]) % VOC[1]
    c = xf % VOC[2]
    ab = (a + 256 * b).astype(np.uint16)
    c = c.astype(np.uint16)

    u0 = np.asarray(U0, dtype=np.float32)
    u1 = np.asarray(U1, dtype=np.float32)
    u2 = np.asarray(U2, dtype=np.float32)
    v0 = np.asarray(V0, dtype=np.float32)
    v1 = np.asarray(V1, dtype=np.float32)
    # B[d*16+e, r] = V0[d,r] * V1[e,r]; btb = B.T replicated at 4
    # partition blocks for the per-pack output matmuls.
    btb = (v0[:, None, :] * v1[None, :, :]).reshape(E, RANK).T  # [32, 128]
    consts = np.zeros((P, 258), dtype=np.float32)
    consts[: VOC[0], 0:32] = u0
    consts[: VOC[1], 32:64] = u1
    consts[: VOC[2], 64:96] = u2
    consts[VOC[2] : 2 * VOC[2], 96:128] = u2
    consts[:, 128:256] = np.tile(btb, (4, 1))
    consts[: VOC[0], 256] = np.arange(VOC[0], dtype=np.float32)
    consts[: VOC[0], 257] = 256.0 * np.arange(VOC[0], dtype=np.float32)

    in_maps = []
    for k in range(N_CORES):
        sl = slice(k * N_CORE, (k + 1) * N_CORE)
        abk = ab[sl]
        ck = c[sl]
        # paired c one-hot plane (built on host; same bytes as replicated
        # c values): one 512-col block per pack pair, rows 0-49 hot for the
        # even pack's c, rows 50-99 for the odd pack's c.
        crep = np.zeros((VOC[0], PAIR_COLS), dtype=ml_dtypes.bfloat16)
        cols = np.arange(PACK)
        pk = 0
        pq = 0
        for sp in SUPERS:
            for q in range((sp + 1) // 2):
                e = ck[(pk + 2 * q) * PACK : (pk + 2 * q + 1) * PACK]
                crep[e, pq * PACK + cols] = 1.0
                if 2 * q + 1 < sp:
                    o = ck[(pk + 2 * q + 1) * PACK : (pk + 2 * q + 2) * PACK]
                    crep[VOC[2] + o, pq * PACK + cols] = 1.0
                pq += 1
            pk += sp
        in_maps.append(
            {
                "abrep": np.ascontiguousarray(
                    np.broadcast_to(abk[None, :], (VOC[0], N_CORE))
                ),
                "crep": crep,
                "consts": consts,
            }
        )
    return in_maps


def kernel(x, U0, U1, U2, V0, V1, _trace=False, _tmpdir=None):
    nc = _get_program()
    in_maps = make_in_maps(x, U0, U1, U2, V0, V1)
    res = bass_utils.run_bass_kernel_spmd(
        nc, in_maps, core_ids=list(range(N_CORES)), trace=_trace, tmpdir=_tmpdir
    )
    out = np.concatenate(
        [
            np.asarray(res.results[k]["out"]).astype(np.float32).T
            for k in range(N_CORES)
        ],
        axis=0,
    )
    out = out.reshape(*np.asarray(x).shape, E)
    if _trace:
        kernel._last_result = res
    return out


# revision 53
# speedup vs baseline: 1.0179x; 1.0179x over previous
"""CP-decomposed embedding lookup kernel for Trainium2 (8 NeuronCores).

Math (matches the CPEmbedding reference):
    A = khatri_rao(U0, U1, U2)            # [500000, 32]
    B = khatri_rao(V0, V1)                # [128, 32]
    out = (A @ B.T)[x]                    # [1024, 200, 128]

Per lookup x = a*5000 + b*50 + c:
    w[r]   = U0[a, r] * U1[b, r] * U2[c, r]
    out[x] = w @ B.T

Instead of per-row DMA gathers (whose SWDGE descriptor generation serializes
on the Q7/Pool engine at ~8 ns/row -> 410 us/core), the factor gathers are
computed as one-hot matmuls on the idle Tensor engine:

    oh_a[v, i] = (a_i == v)   (bf16, exact)     G0T = U0.T @ oh_a  [32, n]
    wT = G0T * G1T * G2T  (DVE elementwise)     out = wT.T @ B.T

Index delivery: the host replicates packed u16 index planes down the
partition axis (p = a + 256*b on 100 partitions), and the a/b one-hot
compares run as DVE tensor_scalar passes from SBUF at 2-byte dtype (fast
DVE mode), with no PSUM broadcast needed. The DVE ISA cannot mix bitwise
and arithmetic ops in one pass (and has no mod), so the field extraction
is a separate bitwise_and pass before each is_equal. The (tiny) c one-hot
plane is built directly on the host - same bytes as replicated c values.

Packing: rank is only 32, so four 512-lookup "packs" share each PSUM bank
at partition offsets 0/32/64/96 (weights loaded at PE array column offsets
via tile_position). The two Khatri-Rao products then run as single
[128, 512] DVE ops covering 2048 lookups each.

c-pairing: the c factor has only 50 vocab rows, so TWO packs' U2 gathers
run as ONE K=100 matmul with a block-diagonal [100, 64] weight (top rows =
U2 for the even pack at rank cols 0-32, bottom rows = U2 for the odd pack
at 32-64). The host lays the replicated c-plane with the partner pack's
c+50 in rows 50-99 so one is_equal against iota(100) produces the stacked
pair one-hot directly.

Output: the final contraction runs TRANSPOSED — one N=512 matmul per pack
(lhsT = the constant B.T rank-block, rhs = the wT slice) producing
[emb, lookups] in PSUM, so each pack needs 1 output matmul instead of 4
and one ldweights of the constant. DRAM output is [128, 25600] bf16
(emb-major); the host transposes and upcasts during assembly.

Sharding: CP factors replicated; the 204800 lookups are split evenly across
the 8 cores (each computes a contiguous [25600, 128] slice of the output).
"""

import ml_dtypes
import numpy as np

import concourse.bacc as bacc
import concourse.bass as bass
import concourse.mybir as mybir
import concourse.tile as tile
from concourse import bass_utils

# Problem constants (hardcoded per the harness contract).
VOC = (100, 100, 50)  # a, b, c
RANK = 32
E = 128  # emb = 8 * 16
N_CORES = 8
X_SHAPE = (1024, 200)
N_TOTAL = X_SHAPE[0] * X_SHAPE[1]  # 204800
N_CORE = N_TOTAL // N_CORES  # 25600
P = 128

PACK = 512  # lookups per pack (one PSUM-bank column span at fp32)
# supers: groups of packs processed per pipeline stage. Small ramp-up supers
# let the PE start after ~150 KB of index DMA instead of 613 KB; a small
# tail super shortens the copy/DMA drain. 50 packs total = 25600 lookups.
SUPERS = [1, 1, 2] + [4] * 11 + [1, 1]  # packs per super
assert sum(SUPERS) * PACK == N_CORE
# c-pair columns: one 512-col block per pack PAIR (odd tail pack gets a
# half-empty block whose bottom rows match nothing).
PAIRS = [(sp + 1) // 2 for sp in SUPERS]
PAIR_COLS = sum(PAIRS) * PACK

F32 = mybir.dt.float32
BF16 = mybir.dt.bfloat16
U16 = mybir.dt.uint16

AND = mybir.AluOpType.bitwise_and
EQ = mybir.AluOpType.is_equal
MULT = mybir.AluOpType.mult


def build_program():
    nc = bacc.Bacc("TRN2", target_bir_lowering=False, debug=False)

    # ---- DRAM I/O (per core) ----
    abrep_d = nc.dram_tensor("abrep", [VOC[0], N_CORE], U16, kind="ExternalInput")
    crep_d = nc.dram_tensor("crep", [VOC[0], PAIR_COLS], BF16, kind="ExternalInput")
    # all factor/iota constants packed in one tensor -> ONE startup DMA:
    # cols 0-31 u0, 32-63 u1, 64-127 u2pair, 128-255 btb4, 256 iota_a,
    # 257 iota_b (= iota_a).
    consts_d = nc.dram_tensor("consts", [P, 258], F32, kind="ExternalInput")
    out_d = nc.dram_tensor("out", [P, N_CORE], BF16, kind="ExternalOutput")

    with tile.TileContext(nc) as tc:
        const = tc.alloc_tile_pool(name="const", bufs=1)

        # ---------- one-time setup ----------
        cf = const.tile([P, 258], F32)
        nc.sync.dma_start(cf[:], consts_d.ap())
        cb = const.tile([P, 256], BF16)
        nc.vector.tensor_copy(cb[:], cf[:][:, 0:256])
        u0b = cb[:][0 : VOC[0], 0:32]
        u1b = cb[:][0 : VOC[1], 32:64]
        u2b = cb[:][0 : VOC[0], 64:128]
        btb = cb[:][:, 128:256]
        iota_a = cf[:][0 : VOC[0], 256:257]
        iota_b = cf[:][0 : VOC[0], 257:258]

        # ---------- pools ----------
        idxp = tc.alloc_tile_pool(name="idx", bufs=4)
        extp = tc.alloc_tile_pool(name="ext", bufs=3)
        ohp = tc.alloc_tile_pool(name="oh", bufs=3)
        wp = tc.alloc_tile_pool(name="w", bufs=2)
        wtp = tc.alloc_tile_pool(name="wt", bufs=2)
        osp = tc.alloc_tile_pool(name="os", bufs=3)
        # PSUM: 3 single-buffered G banks (the DVE products drain them
        # early) + two rotating 2-bank output tiles (pack pairs share one
        # ACT copy) + 1 warmup bank = 8 banks
        gp = tc.alloc_tile_pool(name="g", bufs=1, space="PSUM")
        op = tc.alloc_tile_pool(name="o", bufs=2, space="PSUM")

        MAXS = max(SUPERS) * PACK

        # Per-super state carried between loop iterations for 1-deep
        # software pipelining (o-matmuls of super s emitted after the G
        # matmuls of super s+1 so the PE never waits on the DVE products).
        pend = None  # (wt_tile, n_packs, row0)

        def emit_back_end(pend):
            wt, sp, col0 = pend
            # one SBUF staging tile and ONE output DMA per super: each
            # dma_start costs ~700ns of serial SP-sequencer issue time, so
            # merging 4 per-pack stores into one is a direct wall-clock win.
            osb = osp.tile([P, 4 * PACK], BF16, tag="osb")
            for h in range((sp + 1) // 2):
                w = min(2, sp - 2 * h)  # packs in this pair
                opr = op.tile([P, 2 * PACK], F32, tag="opair")
                for pp in range(w):
                    p = 2 * h + pp
                    # transposed final contraction: out[e, i] for the whole
                    # pack in ONE matmul (constant B.T block stationary).
                    nc.tensor.matmul(
                        out=opr[:][:, pp * PACK : (pp + 1) * PACK],
                        lhsT=btb[32 * p : 32 * p + 32, :],
                        rhs=wt[:][32 * p : 32 * p + 32, :],
                        start=True,
                        stop=True,
                        tile_position=(32 * p, 0),
                    )
                # one ACT copy serves both packs of the pair
                nc.scalar.copy(
                    osb[:][:, 2 * h * PACK : (2 * h + w) * PACK],
                    opr[:][:, 0 : w * PACK],
                )
            nc.sync.dma_start(
                out_d.ap()[:, col0 : col0 + sp * PACK], osb[:][:, 0 : sp * PACK]
            )

        MAXP = max(PAIRS) * PACK
        NS = len(SUPERS)
        offs = [sum(SUPERS[:i]) * PACK for i in range(NS)]
        poffs = [sum(PAIRS[:i]) * PACK for i in range(NS)]

        def emit_front(si):
            """Index DMA + one-hot builds for super si (sync + DVE)."""
            sp = SUPERS[si]
            S = sp * PACK
            S2 = PAIRS[si] * PACK
            off, poff = offs[si], poffs[si]
            abr = idxp.tile([VOC[0], MAXS], U16, tag="abr", name=f"abr{si}")
            # the c one-hot is host-built (same bytes as replicated c values)
            oh_c = ohp.tile([VOC[0], MAXP], BF16, tag="ohc", name=f"ohc{si}")
            nc.sync.dma_start(abr[:][:, 0:S], abrep_d.ap()[:, off : off + S])
            nc.scalar.dma_start(oh_c[:][:, 0:S2], crep_d.ap()[:, poff : poff + S2])

            ta = extp.tile([VOC[0], MAXS], U16, tag="ta", name=f"ta{si}")
            tb = extp.tile([VOC[0], MAXS], U16, tag="tb", name=f"tb{si}")
            nc.vector.tensor_scalar(
                out=ta[:][:, 0:S], in0=abr[:][:, 0:S],
                scalar1=0x00FF, scalar2=None, op0=AND,
            )
            nc.vector.tensor_scalar(
                out=tb[:][:, 0:S], in0=abr[:][:, 0:S],
                scalar1=0xFF00, scalar2=None, op0=AND,
            )
            oh_a = ohp.tile([VOC[0], MAXS], BF16, tag="oha", name=f"oha{si}")
            oh_b = ohp.tile([VOC[1], MAXS], BF16, tag="ohb", name=f"ohb{si}")
            nc.vector.tensor_scalar(
                out=oh_a[:][:, 0:S], in0=ta[:][:, 0:S],
                scalar1=iota_a, scalar2=None, op0=EQ,
            )
            nc.vector.tensor_scalar(
                out=oh_b[:][:, 0:S], in0=tb[:][:, 0:S],
                scalar1=iota_b, scalar2=None, op0=EQ,
            )
            fronts[si] = (
                oh_a[:][:, 0:S],
                oh_b[:][:, 0:S],
                oh_c[:][:, 0:S2],
            )

        def emit_mid(si, front):
            """G matmuls (PE) with the g0 staging copy (ACT) overlapped."""
            sp = SUPERS[si]
            oh_a, oh_b, oh_c = front  # AP views into the group front tiles
            g0 = gp.tile([P, PACK], F32, tag="g0", name=f"g0_{si}")
            g1 = gp.tile([P, PACK], F32, tag="g1", name=f"g1_{si}")
            g2 = gp.tile([P, PACK], F32, tag="g2", name=f"g2_{si}")
            nr = 32 * sp
            s0 = wp.tile([P, PACK], F32, tag="s0", name=f"s0_{si}")
            for p in range(sp):
                nc.tensor.matmul(
                    out=g0[:][32 * p : 32 * p + 32, :],
                    lhsT=u0b, rhs=oh_a[:, p * PACK : (p + 1) * PACK],
                    start=True, stop=True, tile_position=(0, 32 * p),
                )
            # stage g0 -> SBUF on ACT while the PE continues with g1/g2
            # (DVE tensor_tensor may read at most one PSUM operand).
            nc.scalar.copy(s0[:][0:nr, :], g0[:][0:nr, :])
            for p in range(sp):
                nc.tensor.matmul(
                    out=g1[:][32 * p : 32 * p + 32, :],
                    lhsT=u1b, rhs=oh_b[:, p * PACK : (p + 1) * PACK],
                    start=True, stop=True, tile_position=(0, 32 * p),
                )
            for q in range(PAIRS[si]):
                # one K=100 matmul gathers U2 for BOTH packs of the pair
                # (block-diagonal weights; odd tail pair has a zero bottom).
                nc.tensor.matmul(
                    out=g2[:][64 * q : 64 * q + 64, :],
                    lhsT=u2b,
                    rhs=oh_c[:, q * PACK : (q + 1) * PACK],
                    start=True, stop=True, tile_position=(0, 64 * q),
                )
            return g0, g1, g2, s0

        def emit_prods(si, mid):
            sp = SUPERS[si]
            _, g1, g2, s0 = mid
            nr = 32 * sp
            w01 = wp.tile([P, PACK], F32, tag="w01", name=f"w01_{si}")
            wt = wtp.tile([P, PACK], BF16, tag="wt", name=f"wt{si}")
            nc.vector.tensor_tensor(
                out=w01[:][0:nr, :], in0=s0[:][0:nr, :], in1=g1[:][0:nr, :],
                op=MULT,
            )
            nc.vector.tensor_tensor(
                out=wt[:][0:nr, :], in0=w01[:][0:nr, :], in1=g2[:][0:nr, :],
                op=MULT,
            )
            return wt

        # PE warm-up: ~3us of back-to-back junk matmuls during the DMA fill
        # window so the Tensor engine's activity-gated clock ramps to 2.4GHz
        # before the first real matmul (it idles at half clock otherwise).
        for wi in range(7):
            wps = op.tile([P, PACK], F32, tag="warm", name=f"warm{wi}", bufs=1)
            nc.tensor.matmul(
                out=wps[:][:, 0:128], lhsT=cb[:][:, 128:256],
                rhs=cb[:][:, 0:128], start=True, stop=True,
            )

        # Software pipeline: upcoming supers' one-hot builds are queued on
        # the DVE before this super's W-products, so the DVE works ahead
        # while the PE runs G(s) instead of ping-ponging serially.
        fronts = {}
        emit_front(0)
        if NS > 1:
            emit_front(1)
        for si, sp in enumerate(SUPERS):
            if si + 2 < NS:
                emit_front(si + 2)
            mid = emit_mid(si, fronts.pop(si))
            if pend is not None:
                emit_back_end(pend)
            wt = emit_prods(si, mid)
            pend = (wt, sp, offs[si])

        emit_back_end(pend)

        for pool in (op, gp, osp, wtp, wp, ohp, extp, idxp, const):
            pool.release()

    nc.compile()
    return nc


_CACHE: dict = {}


def _get_program():
    if "nc" not in _CACHE:
        _CACHE["nc"] = build_program()
    return _CACHE["nc"]


def make_in_maps(x, U0, U1, U2, V0, V1):
    xf = np.asarray(x).reshape(-1).astype(np.int64)
    a = xf // (VOC[1] * VOC[2])
    b = (xf // VOC[2]) % VOC[1]
    c = xf % VOC[2]
    ab = (a + 256 * b).astype(np.uint16)
    c = c.astype(np.uint16)

    u0 = np.asarray(U0, dtype=np.float32)
    u1 = np.asarray(U1, dtype=np.float32)
    u2 = np.asarray(U2, dtype=np.float32)
    v0 = np.asarray(V0, dtype=np.float32)
    v1 = np.asarray(V1, dtype=np.float32)
    # B[d*16+e, r] = V0[d,r] * V1[e,r]; btb = B.T replicated at 4
    # partition blocks for the per-pack output matmuls.
    btb = (v0[:, None, :] * v1[None, :, :]).reshape(E, RANK).T  # [32, 128]
    consts = np.zeros((P, 258), dtype=np.float32)
    consts[: VOC[0], 0:32] = u0
    consts[: VOC[1], 32:64] = u1
    consts[: VOC[2], 64:96] = u2
    consts[VOC[2] : 2 * VOC[2], 96:128] = u2
    consts[:, 128:256] = np.tile(btb, (4, 1))
    consts[: VOC[0], 256] = np.arange(VOC[0], dtype=np.float32)
    consts[: VOC[0], 257] = 256.0 * np.arange(VOC[0], dtype=np.float32)

    in_maps = []
    for k in range(N_CORES):
        sl = slice(k * N_CORE, (k + 1) * N_CORE)
        abk = ab[sl]
        ck = c[sl]
        # paired c one-hot plane (built on host; same bytes as replicated
        # c values): one 512-col block per pack pair, rows 0-49 hot for the
        # even pack's c, rows 50-99 for the odd pack's c.
        crep = np.zeros((VOC[0], PAIR_COLS), dtype=ml_dtypes.bfloat16)
        cols = np.arange(PACK)
        pk = 0
        pq = 0
        for sp in SUPERS:
            for q in range((sp + 1) // 2):
                e = ck[(pk + 2 * q) * PACK : (pk + 2 * q + 1) * PACK]
                crep[e, pq * PACK + cols] = 1.0
                if 2 * q + 1 < sp:
                    o = ck[(pk + 2 * q + 1) * PACK : (pk + 2 * q + 2) * PACK]
                    crep[VOC[2] + o, pq * PACK + cols] = 1.0
                pq += 1
            pk += sp
        in_maps.append(
            {
                "abrep": np.ascontiguousarray(
                    np.broadcast_to(abk[None, :], (VOC[0], N_CORE))
                ),
                "crep": crep,
                "consts": consts,
            }
        )
    return in_maps


def kernel(x, U0, U1, U2, V0, V1, _trace=False, _tmpdir=None):
    nc = _get_program()
    in_maps = make_in_maps(x, U0, U1, U2, V0, V1)
    res = bass_utils.run_bass_kernel_spmd(
        nc, in_maps, core_ids=list(range(N_CORES)), trace=_trace, tmpdir=_tmpdir
    )
    out = np.concatenate(
        [
            np.asarray(res.results[k]["out"]).astype(np.float32).T
            for k in range(N_CORES)
        ],
        axis=0,
    )
    out = out.reshape(*np.asarray(x).shape, E)
    if _trace:
        kernel._last_result = res
    return out


# revision 54
# speedup vs baseline: 1.0217x; 1.0037x over previous
"""CP-decomposed embedding lookup kernel for Trainium2 (8 NeuronCores).

Math (matches the CPEmbedding reference):
    A = khatri_rao(U0, U1, U2)            # [500000, 32]
    B = khatri_rao(V0, V1)                # [128, 32]
    out = (A @ B.T)[x]                    # [1024, 200, 128]

Per lookup x = a*5000 + b*50 + c:
    w[r]   = U0[a, r] * U1[b, r] * U2[c, r]
    out[x] = w @ B.T

Instead of per-row DMA gathers (whose SWDGE descriptor generation serializes
on the Q7/Pool engine at ~8 ns/row -> 410 us/core), the factor gathers are
computed as one-hot matmuls on the idle Tensor engine:

    oh_a[v, i] = (a_i == v)   (bf16, exact)     G0T = U0.T @ oh_a  [32, n]
    wT = G0T * G1T * G2T  (DVE elementwise)     out = wT.T @ B.T

Index delivery: the host replicates packed u16 index planes down the
partition axis (p = a + 256*b on 100 partitions), and the a/b one-hot
compares run as DVE tensor_scalar passes from SBUF at 2-byte dtype (fast
DVE mode), with no PSUM broadcast needed. The DVE ISA cannot mix bitwise
and arithmetic ops in one pass (and has no mod), so the field extraction
is a separate bitwise_and pass before each is_equal. The (tiny) c one-hot
plane is built directly on the host - same bytes as replicated c values.

Packing: rank is only 32, so four 512-lookup "packs" share each PSUM bank
at partition offsets 0/32/64/96 (weights loaded at PE array column offsets
via tile_position). The two Khatri-Rao products then run as single
[128, 512] DVE ops covering 2048 lookups each.

c-pairing: the c factor has only 50 vocab rows, so TWO packs' U2 gathers
run as ONE K=100 matmul with a block-diagonal [100, 64] weight (top rows =
U2 for the even pack at rank cols 0-32, bottom rows = U2 for the odd pack
at 32-64). The host lays the replicated c-plane with the partner pack's
c+50 in rows 50-99 so one is_equal against iota(100) produces the stacked
pair one-hot directly.

Output: the final contraction runs TRANSPOSED — one N=512 matmul per pack
(lhsT = the constant B.T rank-block, rhs = the wT slice) producing
[emb, lookups] in PSUM, so each pack needs 1 output matmul instead of 4
and one ldweights of the constant. DRAM output is [128, 25600] bf16
(emb-major); the host transposes and upcasts during assembly.

Sharding: CP factors replicated; the 204800 lookups are split evenly across
the 8 cores (each computes a contiguous [25600, 128] slice of the output).
"""

import ml_dtypes
import numpy as np

import concourse.bacc as bacc
import concourse.bass as bass
import concourse.mybir as mybir
import concourse.tile as tile
from concourse import bass_utils

# Problem constants (hardcoded per the harness contract).
VOC = (100, 100, 50)  # a, b, c
RANK = 32
E = 128  # emb = 8 * 16
N_CORES = 8
X_SHAPE = (1024, 200)
N_TOTAL = X_SHAPE[0] * X_SHAPE[1]  # 204800
N_CORE = N_TOTAL // N_CORES  # 25600
P = 128

PACK = 512  # lookups per pack (one PSUM-bank column span at fp32)
# supers: groups of packs processed per pipeline stage. Small ramp-up supers
# let the PE start after ~150 KB of index DMA instead of 613 KB; a small
# tail super shortens the copy/DMA drain. 50 packs total = 25600 lookups.
SUPERS = [1, 1, 2] + [4] * 11 + [1, 1]  # packs per super
assert sum(SUPERS) * PACK == N_CORE
# c-pair columns: one 512-col block per pack PAIR (odd tail pack gets a
# half-empty block whose bottom rows match nothing).
PAIRS = [(sp + 1) // 2 for sp in SUPERS]
PAIR_COLS = sum(PAIRS) * PACK

F32 = mybir.dt.float32
BF16 = mybir.dt.bfloat16
U16 = mybir.dt.uint16

AND = mybir.AluOpType.bitwise_and
EQ = mybir.AluOpType.is_equal
MULT = mybir.AluOpType.mult


def build_program():
    nc = bacc.Bacc("TRN2", target_bir_lowering=False, debug=False)

    # ---- DRAM I/O (per core) ----
    abrep_d = nc.dram_tensor("abrep", [VOC[0], N_CORE], U16, kind="ExternalInput")
    crep_d = nc.dram_tensor("crep", [VOC[0], PAIR_COLS], BF16, kind="ExternalInput")
    # supers 0 and 1 get host-built a/b one-hots (one small DMA each) so
    # the PE starts ~2.5us earlier than the consts->iota->AND->EQ chain.
    ohab01_d = nc.dram_tensor("ohab01", [VOC[0], 2048], BF16, kind="ExternalInput")
    # all factor/iota constants packed in one tensor -> ONE startup DMA:
    # cols 0-31 u0, 32-63 u1, 64-127 u2pair, 128-255 btb4, 256 iota_a,
    # 257 iota_b (= iota_a).
    consts_d = nc.dram_tensor("consts", [P, 258], F32, kind="ExternalInput")
    out_d = nc.dram_tensor("out", [P, N_CORE], BF16, kind="ExternalOutput")

    with tile.TileContext(nc) as tc:
        const = tc.alloc_tile_pool(name="const", bufs=1)

        # ---------- one-time setup ----------
        cf = const.tile([P, 258], F32)
        nc.sync.dma_start(cf[:], consts_d.ap())
        cb = const.tile([P, 256], BF16)
        nc.vector.tensor_copy(cb[:], cf[:][:, 0:256])
        u0b = cb[:][0 : VOC[0], 0:32]
        u1b = cb[:][0 : VOC[1], 32:64]
        u2b = cb[:][0 : VOC[0], 64:128]
        btb = cb[:][:, 128:256]
        iota_a = cf[:][0 : VOC[0], 256:257]
        iota_b = cf[:][0 : VOC[0], 257:258]

        # ---------- pools ----------
        idxp = tc.alloc_tile_pool(name="idx", bufs=4)
        extp = tc.alloc_tile_pool(name="ext", bufs=3)
        ohp = tc.alloc_tile_pool(name="oh", bufs=3)
        wp = tc.alloc_tile_pool(name="w", bufs=2)
        wtp = tc.alloc_tile_pool(name="wt", bufs=2)
        osp = tc.alloc_tile_pool(name="os", bufs=3)
        # PSUM: 3 single-buffered G banks (the DVE products drain them
        # early) + two rotating 2-bank output tiles (pack pairs share one
        # ACT copy) + 1 warmup bank = 8 banks
        gp = tc.alloc_tile_pool(name="g", bufs=1, space="PSUM")
        op = tc.alloc_tile_pool(name="o", bufs=2, space="PSUM")

        MAXS = max(SUPERS) * PACK

        # Per-super state carried between loop iterations for 1-deep
        # software pipelining (o-matmuls of super s emitted after the G
        # matmuls of super s+1 so the PE never waits on the DVE products).
        pend = None  # (wt_tile, n_packs, row0)

        def emit_back_end(pend):
            wt, sp, col0 = pend
            # one SBUF staging tile and ONE output DMA per super: each
            # dma_start costs ~700ns of serial SP-sequencer issue time, so
            # merging 4 per-pack stores into one is a direct wall-clock win.
            osb = osp.tile([P, 4 * PACK], BF16, tag="osb")
            for h in range((sp + 1) // 2):
                w = min(2, sp - 2 * h)  # packs in this pair
                opr = op.tile([P, 2 * PACK], F32, tag="opair")
                for pp in range(w):
                    p = 2 * h + pp
                    # transposed final contraction: out[e, i] for the whole
                    # pack in ONE matmul (constant B.T block stationary).
                    nc.tensor.matmul(
                        out=opr[:][:, pp * PACK : (pp + 1) * PACK],
                        lhsT=btb[32 * p : 32 * p + 32, :],
                        rhs=wt[:][32 * p : 32 * p + 32, :],
                        start=True,
                        stop=True,
                        tile_position=(32 * p, 0),
                    )
                # one ACT copy serves both packs of the pair
                nc.scalar.copy(
                    osb[:][:, 2 * h * PACK : (2 * h + w) * PACK],
                    opr[:][:, 0 : w * PACK],
                )
            nc.sync.dma_start(
                out_d.ap()[:, col0 : col0 + sp * PACK], osb[:][:, 0 : sp * PACK]
            )

        MAXP = max(PAIRS) * PACK
        NS = len(SUPERS)
        offs = [sum(SUPERS[:i]) * PACK for i in range(NS)]
        poffs = [sum(PAIRS[:i]) * PACK for i in range(NS)]

        def emit_front(si):
            """Index DMA + one-hot builds for super si (sync + DVE)."""
            sp = SUPERS[si]
            S = sp * PACK
            S2 = PAIRS[si] * PACK
            off, poff = offs[si], poffs[si]
            # the c one-hot is host-built (same bytes as replicated c values)
            oh_c = ohp.tile([VOC[0], MAXP], BF16, tag="ohc", name=f"ohc{si}")
            nc.scalar.dma_start(oh_c[:][:, 0:S2], crep_d.ap()[:, poff : poff + S2])
            if si < 2:
                # ramp supers: host-built a/b one-hots, no extraction chain
                ohab = idxp.tile([VOC[0], 1024], BF16, tag="ohab", name=f"ohab{si}")
                nc.sync.dma_start(ohab[:], ohab01_d.ap()[:, si * 1024 : si * 1024 + 1024])
                fronts[si] = (
                    ohab[:][:, 0:512],
                    ohab[:][:, 512:1024],
                    oh_c[:][:, 0:S2],
                )
                return
            abr = idxp.tile([VOC[0], MAXS], U16, tag="abr", name=f"abr{si}")
            nc.sync.dma_start(abr[:][:, 0:S], abrep_d.ap()[:, off : off + S])

            ta = extp.tile([VOC[0], MAXS], U16, tag="ta", name=f"ta{si}")
            tb = extp.tile([VOC[0], MAXS], U16, tag="tb", name=f"tb{si}")
            nc.vector.tensor_scalar(
                out=ta[:][:, 0:S], in0=abr[:][:, 0:S],
                scalar1=0x00FF, scalar2=None, op0=AND,
            )
            nc.vector.tensor_scalar(
                out=tb[:][:, 0:S], in0=abr[:][:, 0:S],
                scalar1=0xFF00, scalar2=None, op0=AND,
            )
            oh_a = ohp.tile([VOC[0], MAXS], BF16, tag="oha", name=f"oha{si}")
            oh_b = ohp.tile([VOC[1], MAXS], BF16, tag="ohb", name=f"ohb{si}")
            nc.vector.tensor_scalar(
                out=oh_a[:][:, 0:S], in0=ta[:][:, 0:S],
                scalar1=iota_a, scalar2=None, op0=EQ,
            )
            nc.vector.tensor_scalar(
                out=oh_b[:][:, 0:S], in0=tb[:][:, 0:S],
                scalar1=iota_b, scalar2=None, op0=EQ,
            )
            fronts[si] = (
                oh_a[:][:, 0:S],
                oh_b[:][:, 0:S],
                oh_c[:][:, 0:S2],
            )

        def emit_mid(si, front):
            """G matmuls (PE) with the g0 staging copy (ACT) overlapped."""
            sp = SUPERS[si]
            oh_a, oh_b, oh_c = front  # AP views into the group front tiles
            g0 = gp.tile([P, PACK], F32, tag="g0", name=f"g0_{si}")
            g1 = gp.tile([P, PACK], F32, tag="g1", name=f"g1_{si}")
            g2 = gp.tile([P, PACK], F32, tag="g2", name=f"g2_{si}")
            nr = 32 * sp
            s0 = wp.tile([P, PACK], F32, tag="s0", name=f"s0_{si}")
            for p in range(sp):
                nc.tensor.matmul(
                    out=g0[:][32 * p : 32 * p + 32, :],
                    lhsT=u0b, rhs=oh_a[:, p * PACK : (p + 1) * PACK],
                    start=True, stop=True, tile_position=(0, 32 * p),
                )
            # stage g0 -> SBUF on ACT while the PE continues with g1/g2
            # (DVE tensor_tensor may read at most one PSUM operand).
            nc.scalar.copy(s0[:][0:nr, :], g0[:][0:nr, :])
            for p in range(sp):
                nc.tensor.matmul(
                    out=g1[:][32 * p : 32 * p + 32, :],
                    lhsT=u1b, rhs=oh_b[:, p * PACK : (p + 1) * PACK],
                    start=True, stop=True, tile_position=(0, 32 * p),
                )
            for q in range(PAIRS[si]):
                # one K=100 matmul gathers U2 for BOTH packs of the pair
                # (block-diagonal weights; odd tail pair has a zero bottom).
                nc.tensor.matmul(
                    out=g2[:][64 * q : 64 * q + 64, :],
                    lhsT=u2b,
                    rhs=oh_c[:, q * PACK : (q + 1) * PACK],
                    start=True, stop=True, tile_position=(0, 64 * q),
                )
            return g0, g1, g2, s0

        def emit_prods(si, mid):
            sp = SUPERS[si]
            _, g1, g2, s0 = mid
            nr = 32 * sp
            w01 = wp.tile([P, PACK], F32, tag="w01", name=f"w01_{si}")
            wt = wtp.tile([P, PACK], BF16, tag="wt", name=f"wt{si}")
            nc.vector.tensor_tensor(
                out=w01[:][0:nr, :], in0=s0[:][0:nr, :], in1=g1[:][0:nr, :],
                op=MULT,
            )
            nc.vector.tensor_tensor(
                out=wt[:][0:nr, :], in0=w01[:][0:nr, :], in1=g2[:][0:nr, :],
                op=MULT,
            )
            return wt

        # PE warm-up: ~3us of back-to-back junk matmuls during the DMA fill
        # window so the Tensor engine's activity-gated clock ramps to 2.4GHz
        # before the first real matmul (it idles at half clock otherwise).
        for wi in range(7):
            wps = op.tile([P, PACK], F32, tag="warm", name=f"warm{wi}", bufs=1)
            nc.tensor.matmul(
                out=wps[:][:, 0:128], lhsT=cb[:][:, 128:256],
                rhs=cb[:][:, 0:128], start=True, stop=True,
            )

        # Software pipeline: upcoming supers' one-hot builds are queued on
        # the DVE before this super's W-products, so the DVE works ahead
        # while the PE runs G(s) instead of ping-ponging serially.
        fronts = {}
        emit_front(0)
        if NS > 1:
            emit_front(1)
        for si, sp in enumerate(SUPERS):
            if si + 2 < NS:
                emit_front(si + 2)
            mid = emit_mid(si, fronts.pop(si))
            if pend is not None:
                emit_back_end(pend)
            wt = emit_prods(si, mid)
            pend = (wt, sp, offs[si])

        emit_back_end(pend)

        for pool in (op, gp, osp, wtp, wp, ohp, extp, idxp, const):
            pool.release()

    nc.compile()
    return nc


_CACHE: dict = {}


def _get_program():
    if "nc" not in _CACHE:
        _CACHE["nc"] = build_program()
    return _CACHE["nc"]


def make_in_maps(x, U0, U1, U2, V0, V1):
    xf = np.asarray(x).reshape(-1).astype(np.int64)
    a = xf // (VOC[1] * VOC[2])
    b = (xf // VOC[2]) % VOC[1]
    c = xf % VOC[2]
    ab = (a + 256 * b).astype(np.uint16)
    c = c.astype(np.uint16)

    u0 = np.asarray(U0, dtype=np.float32)
    u1 = np.asarray(U1, dtype=np.float32)
    u2 = np.asarray(U2, dtype=np.float32)
    v0 = np.asarray(V0, dtype=np.float32)
    v1 = np.asarray(V1, dtype=np.float32)
    # B[d*16+e, r] = V0[d,r] * V1[e,r]; btb = B.T replicated at 4
    # partition blocks for the per-pack output matmuls.
    btb = (v0[:, None, :] * v1[None, :, :]).reshape(E, RANK).T  # [32, 128]
    consts = np.zeros((P, 258), dtype=np.float32)
    consts[: VOC[0], 0:32] = u0
    consts[: VOC[1], 32:64] = u1
    consts[: VOC[2], 64:96] = u2
    consts[VOC[2] : 2 * VOC[2], 96:128] = u2
    consts[:, 128:256] = np.tile(btb, (4, 1))
    consts[: VOC[0], 256] = np.arange(VOC[0], dtype=np.float32)
    consts[: VOC[0], 257] = 256.0 * np.arange(VOC[0], dtype=np.float32)

    in_maps = []
    for k in range(N_CORES):
        sl = slice(k * N_CORE, (k + 1) * N_CORE)
        abk = ab[sl]
        ck = c[sl]
        # paired c one-hot plane (built on host; same bytes as replicated
        # c values): one 512-col block per pack pair, rows 0-49 hot for the
        # even pack's c, rows 50-99 for the odd pack's c.
        crep = np.zeros((VOC[0], PAIR_COLS), dtype=ml_dtypes.bfloat16)
        cols = np.arange(PACK)
        pk = 0
        pq = 0
        for sp in SUPERS:
            for q in range((sp + 1) // 2):
                e = ck[(pk + 2 * q) * PACK : (pk + 2 * q + 1) * PACK]
                crep[e, pq * PACK + cols] = 1.0
                if 2 * q + 1 < sp:
                    o = ck[(pk + 2 * q + 1) * PACK : (pk + 2 * q + 2) * PACK]
                    crep[VOC[2] + o, pq * PACK + cols] = 1.0
                pq += 1
            pk += sp
        ohab01 = np.zeros((VOC[0], 2048), dtype=ml_dtypes.bfloat16)
        c512 = np.arange(512)
        for si in range(2):
            blk = abk[si * 512 : (si + 1) * 512].astype(np.int64)
            ohab01[blk & 0xFF, si * 1024 + c512] = 1.0
            ohab01[(blk >> 8), si * 1024 + 512 + c512] = 1.0
        in_maps.append(
            {
                "abrep": np.ascontiguousarray(
                    np.broadcast_to(abk[None, :], (VOC[0], N_CORE))
                ),
                "ohab01": ohab01,
                "crep": crep,
                "consts": consts,
            }
        )
    return in_maps


def kernel(x, U0, U1, U2, V0, V1, _trace=False, _tmpdir=None):
    nc = _get_program()
    in_maps = make_in_maps(x, U0, U1, U2, V0, V1)
    res = bass_utils.run_bass_kernel_spmd(
        nc, in_maps, core_ids=list(range(N_CORES)), trace=_trace, tmpdir=_tmpdir
    )
    out = np.concatenate(
        [
            np.asarray(res.results[k]["out"]).astype(np.float32).T
            for k in range(N_CORES)
        ],
        axis=0,
    )
    out = out.reshape(*np.asarray(x).shape, E)
    if _trace:
        kernel._last_result = res
    return out
